# revision 47
# baseline (speedup 1.0000x reference)
# Trainium2 Bass kernel for the 4-branch cross-attention block.
#
# Problem: N=4 batches, L1=L2=1024, D=512, H=8 heads of 64.
#   q1,k1,v1 = proj(input1); q2,k2,v2 = proj(input2)
#   four attention branches (q1k1v1, q1k2v2, q2k1v1, q2k2v2), masked softmax
#   over the key axis, outputs averaged pairwise.
#
# Sharding: 8 cores = 4 batches x 2 head-groups (4 heads each). SPMD — one
# program, per-core data.
#
# Design notes (395 us baseline -> ~195 us):
#  - The scalar engine's exp over 16.8M logits (~141 us back-to-back) is the
#    hard floor; everything else is structured to keep the scalar engine
#    saturated and the PE continuously busy (the PE p-state drops from
#    2.4 GHz to 1.2 GHz on idle gaps, which is what made the baseline slow).
#  - Attention runs in the transposed "ST" layout (keys on partitions):
#      ST = K @ Q^T, P = exp(ST + key_mask_bias), O^T = [V|1]^T @ P
#    (the ones column yields the softmax denominators in acc row 64).
#  - One continuous software pipeline across all (branch, kt) steps:
#    QK(n) || exp(n-1..n) || PV(n-1), no per-branch refill bubbles. Both
#    q-sides stream against one kz stationary per (head, kt).
#  - Host pre-casts x and W to fp16 (no device-side CASTs, half the DMA).
#  - kz (zero-padded per-head K stationaries) is written directly from the
#    k-projection PSUM as 4 big [64,1024] copies per side.
#  - ks=2 projections are injected into the pipeline's PE slack as fillers.
#  - Normalization: denominator row rides along the acc->SBUF copy (frees
#    the PSUM acc ring fast), the reciprocal runs in a [128,8] layout (DVE
#    recip cost scales with free size, hence the DMA reshape round trip),
#    the 1/s row is broadcast over 64 partitions by a GPSIMD
#    partition_broadcast on the otherwise-idle Pool engine (a PE rank-1
#    matmul broadcast entangles the PSUM st ring; a stride-0-source DMA
#    broadcast is ~10 us — both measurably worse), and the final muls/adds
#    run on DVE, deferred to the next branch's kt5 where the DVE queue is
#    provably drained.

import sys

sys.path.insert(0, "/opt/trn_rl_repo")

import numpy as np

import concourse.bacc as bacc
import concourse.mybir as mybir
import concourse.tile as tile
from concourse.bass_utils import run_bass_kernel_spmd

F32 = mybir.dt.float32
F32R = mybir.dt.float32r
F16 = mybir.dt.float16
BF16 = mybir.dt.bfloat16
EXP = mybir.ActivationFunctionType.Exp

L = 1024  # sequence length (both sides)
D = 512  # hidden
NB = 4  # batches
HPG = 4  # heads per core (head group)
HD = 64  # head size
OG = HPG * HD  # output channels per core = 256
KT = L // 128  # 8 key tiles
DT = D // 128  # 4 contraction tiles for projections
INF = 10000.0

_NC = None  # cached compiled program
TRACE = False  # set by test harness to capture an NTFF profile
LAST_RESULT = None  # full BassKernelResults of the last run (for profiling)


def _tt(pool, shape, dtype, tag):
    return pool.tile(shape, dtype, tag=tag, name=tag)


def _install_ntff_hook():
    # antenv.axon_hooks is absent in this image; provide it so
    # run_bass_kernel_spmd(trace=True) can capture NTFF profiles.
    import types, contextlib, ctypes

    if "antenv.axon_hooks" in sys.modules:
        return
    lib = ctypes.CDLL("/opt/axon/libaxon_pjrt.so")
    lib.axon_start_nrt_profile.argtypes = [
        ctypes.POINTER(ctypes.c_int64),
        ctypes.c_size_t,
    ]
    lib.axon_start_nrt_profile.restype = ctypes.c_int64
    lib.axon_stop_nrt_profile.argtypes = [ctypes.c_char_p]
    lib.axon_stop_nrt_profile.restype = ctypes.c_int64

    @contextlib.contextmanager
    def _hook(output_dir, device_ids):
        import jax

        jax.devices()
        if device_ids:
            ids = (ctypes.c_int64 * len(device_ids))(*device_ids)
            rc = lib.axon_start_nrt_profile(ids, len(device_ids))
        else:
            rc = lib.axon_start_nrt_profile(None, 0)
        if rc != 0:
            raise RuntimeError(f"axon_start_nrt_profile rc={rc}")
        try:
            yield
        finally:
            n = lib.axon_stop_nrt_profile(str(output_dir).encode())
            print(f"ntff profile: {n} file(s) in {output_dir}", file=sys.stderr)

    mod = types.ModuleType("antenv.axon_hooks")
    mod.get_axon_ntff_profile_hook = lambda: _hook
    mod.set_axon_ntff_profile_hook = lambda h: None
    sys.modules["antenv.axon_hooks"] = mod


def _build():
    nc = bacc.Bacc("TRN2", target_bir_lowering=False, debug=False, num_devices=8)

    x1T = nc.declare_dram_parameter("x1T", [D, L], F16, isOutput=False)
    x2T = nc.declare_dram_parameter("x2T", [D, L], F16, isOutput=False)
    ws = {}
    for wn in ("wq1", "wk1", "wv1", "wq2", "wk2", "wv2"):
        ws[wn] = nc.declare_dram_parameter(wn, [D, OG], F16, isOutput=False)
    bias1 = nc.declare_dram_parameter("bias1", [128, KT], F32, isOutput=False)
    bias2 = nc.declare_dram_parameter("bias2", [128, KT], F32, isOutput=False)
    hm1 = nc.declare_dram_parameter("hm1", [128, KT], F32, isOutput=False)
    hm2 = nc.declare_dram_parameter("hm2", [128, KT], F32, isOutput=False)
    out1T = nc.declare_dram_parameter("out1T", [OG, L], F32, isOutput=True)
    out2T = nc.declare_dram_parameter("out2T", [OG, L], F32, isOutput=True)

    with tile.TileContext(nc) as tc:
        with (
            tc.tile_pool(name="pers", bufs=1) as pers,
            tc.tile_pool(name="pt", bufs=4) as ptp,
            tc.tile_pool(name="oTs", bufs=4) as oTsp,
            tc.tile_pool(name="sm", bufs=2) as smp,
            tc.tile_pool(name="tmp", bufs=2) as tmpp,
            tc.tile_pool(name="st", bufs=2, space="PSUM") as stp,
            tc.tile_pool(name="acc", bufs=2, space="PSUM") as accp,
        ):
            # ---- input DMAs (already f16 on host), in first-use order so
            # the k1 projection can start after the first 8 transfers ----
            x_r = {1: [], 2: []}
            w_r = {wn: [] for wn in ws}

            def load_x(side):
                dram = {1: x1T, 2: x2T}[side]
                for dk in range(DT):
                    t = _tt(pers, [128, L], F16, f"x{side}_{dk}")
                    nc.sync.dma_start(t[:], dram[dk * 128 : (dk + 1) * 128, :])
                    x_r[side].append(t)

            def load_w(wn, eng):
                for dk in range(DT):
                    t = _tt(pers, [128, OG], F16, f"{wn}_{dk}")
                    eng.dma_start(t[:], ws[wn][dk * 128 : (dk + 1) * 128, :])
                    w_r[wn].append(t)

            # descriptor issue is ~650 ns each and strictly serial per
            # queue; split across the two HWDGE queues (sync + scalar — the
            # scalar engine is idle until the first exp) so the first
            # projection's operands land ~5 us sooner.
            load_x(1)
            load_w("wk1", nc.sync)
            b_sb = {}
            for qs, dram in ((1, bias1), (2, bias2)):
                b = _tt(pers, [128, KT], F32, f"bias{qs}")
                nc.sync.dma_start(b[:], dram[:])
                b_sb[qs] = b
            load_w("wq1", nc.sync)
            load_x(2)
            load_w("wq2", nc.sync)
            load_w("wv1", nc.sync)
            load_w("wk2", nc.sync)
            load_w("wv2", nc.sync)
            hm_sb = {}
            for qs, dram in ((1, hm1), (2, hm2)):
                h = _tt(pers, [128, KT], F32, f"hm{qs}")
                nc.sync.dma_start(h[:], dram[:])
                hm_sb[qs] = h
            # ---- persistent device tensors ----
            # kz: zero-padded per-(head, kt) K stationaries, [128, 4*8*128]
            kz = {}
            for ks in (1, 2):
                z = _tt(pers, [128, HPG * KT * 128], F16, f"kz{ks}")
                nc.vector.memset(z[:], 0.0)
                kz[ks] = z
            # qT: [og, L] moving operands, 2 tiles of [128, L] per side
            qT = {1: [], 2: []}
            # v in natural layout with ones column: [128, HPG, 65] per l-tile
            v_e = {1: [], 2: []}
            # output accumulators (SBUF, written by Pool)
            outacc = {
                qs: [_tt(pers, [HD, L], F32, f"out{qs}_{i}") for i in range(HPG)]
                for qs in (1, 2)
            }

            # ---- projection emitters ----
            def proj_k(ks, ots=(0, 1)):
                w = w_r[f"wk{ks}"]
                for ot in ots:
                    ps = _tt(stp, [128, L], F32, "st")
                    for dk in range(DT):
                        for nh in range(2):
                            nc.tensor.matmul(
                                ps[:, nh * 512 : (nh + 1) * 512],
                                w[dk][:, ot * 128 : (ot + 1) * 128],
                                x_r[ks][dk][:, nh * 512 : (nh + 1) * 512],
                                start=(dk == 0),
                                stop=(dk == DT - 1),
                            )
                    # scatter straight into kz: head 2*ot rows 0:64, head
                    # 2*ot+1 rows 64:128, each a contiguous [64, 1024] block.
                    for half in range(2):
                        hh = 2 * ot + half
                        po = half * 64
                        nc.vector.tensor_copy(
                            kz[ks][po : po + 64, hh * L : (hh + 1) * L],
                            ps[po : po + 64, :],
                        )

            def proj_q(qs, ots=(0, 1)):
                w = w_r[f"wq{qs}"]
                for ot in ots:
                    ps = _tt(stp, [128, L], F32, "st")
                    for dk in range(DT):
                        for nh in range(2):
                            nc.tensor.matmul(
                                ps[:, nh * 512 : (nh + 1) * 512],
                                w[dk][:, ot * 128 : (ot + 1) * 128],
                                x_r[qs][dk][:, nh * 512 : (nh + 1) * 512],
                                start=(dk == 0),
                                stop=(dk == DT - 1),
                            )
                    t = _tt(pers, [128, L], F16, f"q{qs}T_{ot}")
                    nc.vector.tensor_copy(t[:], ps[:])
                    qT[qs].append(t)

            def proj_v(side, lts):
                w = w_r[f"wv{side}"]
                for lt in lts:
                    ps = _tt(stp, [128, L], F32, "st")
                    for dk in range(DT):
                        nc.tensor.matmul(
                            ps[:, 0:OG],
                            x_r[side][dk][:, lt * 128 : (lt + 1) * 128],
                            w[dk][:],
                            start=(dk == 0),
                            stop=(dk == DT - 1),
                        )
                    t = _tt(pers, [128, HPG, HD + 1], BF16, f"v{side}_{lt}")
                    nc.vector.tensor_copy(
                        t[:, :, 0:HD], ps[:, 0:OG].rearrange("p (h d) -> p h d", h=HPG)
                    )
                    nc.vector.memset(t[:, :, HD : HD + 1], 1.0)
                    v_e[side].append(t)

            # ---- attention: one continuous software pipeline across all
            # (branch, kt) steps: QK(n) || exp(n-ish) || PV(n-1), so the PE
            # never sees a branch-boundary refill bubble. ----
            def norm_front(h, ks, acc):
                # free the acc banks quickly by copying [o; s] to SBUF (both
                # copies BEFORE the DMA-blocked reciprocal chain), then run
                # the reciprocal in a [128, 8] layout. fast=True (final
                # branch) replaces the two DMA-reshape round trips with a
                # direct [1, L] reciprocal_approx_fast (~18 correct bits) to
                # shorten the tail chain.
                oTs = {}
                srm = {}
                s128s = {}
                qlist = sorted(acc)
                for qs in qlist:
                    o = _tt(oTsp, [HD + 1, L], F32, "oTs")
                    nc.vector.tensor_copy(o[:], acc[qs][:])
                    oTs[qs] = o
                    s128 = _tt(smp, [128, KT], F32, "s128")
                    nc.sync.dma_start(s128[:], o[HD : HD + 1, :])
                    s128s[qs] = s128
                bcs = {}
                for qs in qlist:
                    r128 = _tt(smp, [128, KT], F32, "r128")
                    nc.vector.reciprocal(r128[:], s128s[qs][:])
                    rm128 = _tt(smp, [128, KT], F32, "rm128")
                    nc.vector.tensor_mul(rm128[:], r128[:], hm_sb[qs][:])
                    sr = _tt(smp, [1, L], F32, "srm")
                    nc.sync.dma_start(sr[:], rm128[:])
                    srm[qs] = sr
                    # broadcast 0.5*mask/s over 64 partitions on the idle
                    # Pool engine — keeps the broadcast off the PE and out
                    # of the PSUM st ring.
                    bc = _tt(smp, [HD, L], F32, "bc")
                    nc.gpsimd.partition_broadcast(bc[:], sr[:], channels=HD)
                    bcs[qs] = bc

                def finish():
                    # per-branch output mul/accumulate, qs=1 on DVE and
                    # qs=2 on Pool so the two chains run in parallel (all
                    # operands are SBUF now). Deferred to the next branch's
                    # kt5 so the s -> 1/s -> broadcast chain latency is
                    # hidden.
                    for qs in qlist:
                        oslice = outacc[qs][h][:]
                        if ks == 1:
                            nc.vector.tensor_mul(oslice, oTs[qs][0:HD, :], bcs[qs][:])
                        else:
                            t = _tt(tmpp, [HD, L], F32, "tmp")
                            nc.vector.tensor_mul(t[:], oTs[qs][0:HD, :], bcs[qs][:])
                            nc.vector.tensor_add(oslice, oslice, t[:])
                            nc.sync.dma_start(
                                {1: out1T, 2: out2T}[qs][h * HD : (h + 1) * HD, :],
                                oslice,
                            )

                return finish

            def attention(sched, fillers=()):
                # fillers: emission closures (deferred projection chunks)
                # injected at specific step indices into the pipeline's PE
                # slack. sched entries are (h, ks, qlist); the first real
                # branch is split into q1-only/q2-only passes so attention
                # starts right after the k1+q1 projections.
                accs = {}
                prev = None
                pending = None
                fill = dict(fillers)
                steps = [
                    (bi, h, ks, kt, qls)
                    for bi, (h, ks, qls) in enumerate(sched)
                    for kt in range(KT)
                ]
                for n, (bi, h, ks, kt, qls) in enumerate(steps):
                    if n in fill:
                        fill.pop(n)()
                    if kt == 0:
                        accs[bi] = {
                            qs: _tt(accp, [HD + 1, L], F32, "acc") for qs in qls
                        }
                    blk = h * KT + kt
                    lhsT = kz[ks][:, blk * 128 : (blk + 1) * 128]
                    sts = {}
                    for qs in qls:
                        st = _tt(stp, [128, L], F32, "st")
                        for nh in range(2):
                            nc.tensor.matmul(
                                st[:, nh * 512 : (nh + 1) * 512],
                                lhsT,
                                qT[qs][h // 2][:, nh * 512 : (nh + 1) * 512],
                                start=True,
                                stop=True,
                            )
                        sts[qs] = st
                    pts = {}
                    for qs in qls:
                        pt = _tt(ptp, [128, L], BF16, "pt")
                        nc.scalar.activation(
                            pt[:], sts[qs][:], EXP, bias=b_sb[ks][:, kt : kt + 1]
                        )
                        pts[qs] = pt
                    if prev is not None:
                        pending = _retire(prev, accs, pending)
                    prev = (bi, h, ks, kt, pts)
                pending = _retire(prev, accs, pending)
                if pending is not None:
                    pending()

            def _retire(prev, accs, pending):
                bi, h, ks, kt, pts = prev
                vt = v_e[ks][kt][:, h, :]
                for qs in pts:
                    for nh in range(2):
                        nc.tensor.matmul(
                            accs[bi][qs][:, nh * 512 : (nh + 1) * 512],
                            vt,
                            pts[qs][:, nh * 512 : (nh + 1) * 512],
                            start=(kt == 0),
                            stop=(kt == KT - 1),
                        )
                if kt == KT - 1:
                    return norm_front(h, ks, accs.pop(bi))
                if kt == 5 and pending is not None:
                    # mid-branch finish emission: the DVE queue has drained
                    # the previous norm chain by kt5, so the bc st-ring slots
                    # are consumed immediately, unlike at a branch boundary.
                    pending()
                    return None
                return pending

            # ---- emission schedule: ks=1 operands projected up front; the
            # ks=2 projections are injected into the attention pipeline's
            # slack at fixed step indices (all done well before step 32) ----
            proj_k(1)
            proj_q(1)
            proj_q(2)
            proj_v(1, range(KT))

            fillers = {2: lambda: proj_k(2, (0,)), 6: lambda: proj_k(2, (1,))}
            for j, lt in enumerate(range(KT)):
                fillers[10 + 2 * j] = lambda lt=lt: proj_v(2, (lt,))

            sched = [(h, ks, (1, 2)) for ks in (1, 2) for h in range(HPG)]
            attention(sched, fillers)

    nc.compile()
    return nc


def kernel(**inputs):
    global _NC
    if _NC is None:
        _NC = _build()

    input1 = np.asarray(inputs["input1"], dtype=np.float32)
    input2 = np.asarray(inputs["input2"], dtype=np.float32)
    mask1 = np.asarray(inputs["mask1"], dtype=np.float32)
    mask2 = np.asarray(inputs["mask2"], dtype=np.float32)
    W = {k: np.asarray(inputs[k], dtype=np.float32) for k in
         ("Wq1", "Wk1", "Wv1", "Wq2", "Wk2", "Wv2")}

    in_maps = []
    for core in range(8):
        b, hg = core // 2, core % 2
        og = slice(hg * OG, (hg + 1) * OG)
        m = {
            "x1T": np.ascontiguousarray(input1[b].T.astype(np.float16)),
            "x2T": np.ascontiguousarray(input2[b].T.astype(np.float16)),
            "bias1": np.ascontiguousarray(
                ((mask1[b] - 1.0) * INF).reshape(KT, 128).T
            ),
            "bias2": np.ascontiguousarray(
                ((mask2[b] - 1.0) * INF).reshape(KT, 128).T
            ),
            # [128, 8] layout matching the s-row DMA reshape (partition-major)
            "hm1": np.ascontiguousarray((0.5 * mask1[b]).reshape(128, KT)),
            "hm2": np.ascontiguousarray((0.5 * mask2[b]).reshape(128, KT)),
        }
        for wn in ("q1", "k1", "v1", "q2", "k2", "v2"):
            m["w" + wn] = np.ascontiguousarray(
                W["W" + wn[0] + wn[1]].T[:, og].astype(np.float16)
            )
        in_maps.append(m)

    global LAST_RESULT
    if TRACE:
        _install_ntff_hook()
    res = run_bass_kernel_spmd(_NC, in_maps, list(range(8)), trace=TRACE)
    LAST_RESULT = res

    output1 = np.empty((NB, L, D), dtype=np.float32)
    output2 = np.empty((NB, L, D), dtype=np.float32)
    for core in range(8):
        b, hg = core // 2, core % 2
        og = slice(hg * OG, (hg + 1) * OG)
        output1[b, :, og] = res.results[core]["out1T"].T
        output2[b, :, og] = res.results[core]["out2T"].T
    return (output1, output2)


# revision 48
# speedup vs baseline: 1.1835x; 1.1835x over previous
# Trainium2 Bass kernel for the 4-branch cross-attention block.
#
# Problem: N=4 batches, L1=L2=1024, D=512, H=8 heads of 64.
#   q1,k1,v1 = proj(input1); q2,k2,v2 = proj(input2)
#   four attention branches (q1k1v1, q1k2v2, q2k1v1, q2k2v2), masked softmax
#   over the key axis, outputs averaged pairwise.
#
# Sharding: 8 cores = 4 batches x 2 head-groups (4 heads each). SPMD — one
# program, per-core data.
#
# Design notes (395 us baseline -> ~195 us):
#  - The scalar engine's exp over 16.8M logits (~141 us back-to-back) is the
#    hard floor; everything else is structured to keep the scalar engine
#    saturated and the PE continuously busy (the PE p-state drops from
#    2.4 GHz to 1.2 GHz on idle gaps, which is what made the baseline slow).
#  - Attention runs in the transposed "ST" layout (keys on partitions):
#      ST = K @ Q^T, P = exp(ST + key_mask_bias), O^T = [V|1]^T @ P
#    (the ones column yields the softmax denominators in acc row 64).
#  - One continuous software pipeline across all (branch, kt) steps:
#    QK(n) || exp(n-1..n) || PV(n-1), no per-branch refill bubbles. Both
#    q-sides stream against one kz stationary per (head, kt).
#  - Host pre-casts x and W to fp16 (no device-side CASTs, half the DMA).
#  - kz (zero-padded per-head K stationaries) is written directly from the
#    k-projection PSUM as 4 big [64,1024] copies per side.
#  - ks=2 projections are injected into the pipeline's PE slack as fillers.
#  - Normalization: denominator row rides along the acc->SBUF copy (frees
#    the PSUM acc ring fast), the reciprocal runs in a [128,8] layout (DVE
#    recip cost scales with free size, hence the DMA reshape round trip),
#    the 1/s row is broadcast over 64 partitions by a GPSIMD
#    partition_broadcast on the otherwise-idle Pool engine (a PE rank-1
#    matmul broadcast entangles the PSUM st ring; a stride-0-source DMA
#    broadcast is ~10 us — both measurably worse), and the final muls/adds
#    run on DVE, deferred to the next branch's kt5 where the DVE queue is
#    provably drained.

import sys

sys.path.insert(0, "/opt/trn_rl_repo")

import numpy as np

import concourse.bacc as bacc
import concourse.mybir as mybir
import concourse.tile as tile
from concourse.bass_utils import run_bass_kernel_spmd

F32 = mybir.dt.float32
F32R = mybir.dt.float32r
F16 = mybir.dt.float16
BF16 = mybir.dt.bfloat16
EXP = mybir.ActivationFunctionType.Exp

L = 1024  # sequence length (both sides)
D = 512  # hidden
NB = 4  # batches
HPG = 4  # heads per core (head group)
HD = 64  # head size
OG = HPG * HD  # output channels per core = 256
KT = L // 128  # 8 key tiles
DT = D // 128  # 4 contraction tiles for projections
INF = 10000.0

_NC = None  # cached compiled program
TRACE = False  # set by test harness to capture an NTFF profile
LAST_RESULT = None  # full BassKernelResults of the last run (for profiling)


def _tt(pool, shape, dtype, tag):
    return pool.tile(shape, dtype, tag=tag, name=tag)


def _install_ntff_hook():
    # antenv.axon_hooks is absent in this image; provide it so
    # run_bass_kernel_spmd(trace=True) can capture NTFF profiles.
    import types, contextlib, ctypes

    if "antenv.axon_hooks" in sys.modules:
        return
    lib = ctypes.CDLL("/opt/axon/libaxon_pjrt.so")
    lib.axon_start_nrt_profile.argtypes = [
        ctypes.POINTER(ctypes.c_int64),
        ctypes.c_size_t,
    ]
    lib.axon_start_nrt_profile.restype = ctypes.c_int64
    lib.axon_stop_nrt_profile.argtypes = [ctypes.c_char_p]
    lib.axon_stop_nrt_profile.restype = ctypes.c_int64

    @contextlib.contextmanager
    def _hook(output_dir, device_ids):
        import jax

        jax.devices()
        if device_ids:
            ids = (ctypes.c_int64 * len(device_ids))(*device_ids)
            rc = lib.axon_start_nrt_profile(ids, len(device_ids))
        else:
            rc = lib.axon_start_nrt_profile(None, 0)
        if rc != 0:
            raise RuntimeError(f"axon_start_nrt_profile rc={rc}")
        try:
            yield
        finally:
            n = lib.axon_stop_nrt_profile(str(output_dir).encode())
            print(f"ntff profile: {n} file(s) in {output_dir}", file=sys.stderr)

    mod = types.ModuleType("antenv.axon_hooks")
    mod.get_axon_ntff_profile_hook = lambda: _hook
    mod.set_axon_ntff_profile_hook = lambda h: None
    sys.modules["antenv.axon_hooks"] = mod


def _build():
    nc = bacc.Bacc("TRN2", target_bir_lowering=False, debug=False, num_devices=8)

    x1T = nc.declare_dram_parameter("x1T", [D, L], F16, isOutput=False)
    x2T = nc.declare_dram_parameter("x2T", [D, L], F16, isOutput=False)
    ws = {}
    for wn in ("wq1", "wk1", "wv1", "wq2", "wk2", "wv2"):
        ws[wn] = nc.declare_dram_parameter(wn, [D, OG], F16, isOutput=False)
    bias1 = nc.declare_dram_parameter("bias1", [128, KT], F32, isOutput=False)
    bias2 = nc.declare_dram_parameter("bias2", [128, KT], F32, isOutput=False)
    hm1 = nc.declare_dram_parameter("hm1", [128, KT], F32, isOutput=False)
    hm2 = nc.declare_dram_parameter("hm2", [128, KT], F32, isOutput=False)
    out1T = nc.declare_dram_parameter("out1T", [OG, L], F32, isOutput=True)
    out2T = nc.declare_dram_parameter("out2T", [OG, L], F32, isOutput=True)

    with tile.TileContext(nc) as tc:
        with (
            tc.tile_pool(name="pers", bufs=1) as pers,
            tc.tile_pool(name="pt", bufs=4) as ptp,
            tc.tile_pool(name="oTs", bufs=4) as oTsp,
            tc.tile_pool(name="sm", bufs=2) as smp,
            tc.tile_pool(name="tmp", bufs=2) as tmpp,
            tc.tile_pool(name="st", bufs=2, space="PSUM") as stp,
            tc.tile_pool(name="acc", bufs=2, space="PSUM") as accp,
        ):
            # ---- input DMAs (already f16 on host), in first-use order so
            # the k1 projection can start after the first 8 transfers ----
            x_r = {1: [], 2: []}
            w_r = {wn: [] for wn in ws}

            def load_x(side):
                dram = {1: x1T, 2: x2T}[side]
                for dk in range(DT):
                    t = _tt(pers, [128, L], F16, f"x{side}_{dk}")
                    nc.sync.dma_start(t[:], dram[dk * 128 : (dk + 1) * 128, :])
                    x_r[side].append(t)

            def load_w(wn, eng):
                for dk in range(DT):
                    t = _tt(pers, [128, OG], F16, f"{wn}_{dk}")
                    eng.dma_start(t[:], ws[wn][dk * 128 : (dk + 1) * 128, :])
                    w_r[wn].append(t)

            # descriptor issue is ~650 ns each and strictly serial per
            # queue; split across the two HWDGE queues (sync + scalar — the
            # scalar engine is idle until the first exp) so the first
            # projection's operands land ~5 us sooner.
            load_x(1)
            load_w("wk1", nc.sync)
            load_w("wq1", nc.sync)
            load_x(2)
            load_w("wq2", nc.sync)
            load_w("wv1", nc.sync)
            load_w("wk2", nc.sync)
            load_w("wv2", nc.sync)

            b_sb = {}
            for qs, dram in ((1, bias1), (2, bias2)):
                b = _tt(pers, [128, KT], F32, f"bias{qs}")
                nc.sync.dma_start(b[:], dram[:])
                b_sb[qs] = b
            hm_sb = {}
            for qs, dram in ((1, hm1), (2, hm2)):
                h = _tt(pers, [128, KT], F32, f"hm{qs}")
                nc.sync.dma_start(h[:], dram[:])
                hm_sb[qs] = h
            # ---- persistent device tensors ----
            # kz: zero-padded per-(head, kt) K stationaries, [128, 4*8*128]
            kz = {}
            for ks in (1, 2):
                z = _tt(pers, [128, HPG * KT * 128], F16, f"kz{ks}")
                nc.vector.memset(z[:], 0.0)
                kz[ks] = z
            # qT: [og, L] moving operands, 2 tiles of [128, L] per side
            qT = {1: [], 2: []}
            # v in natural layout with ones column: [128, HPG, 65] per l-tile
            v_e = {1: [], 2: []}
            # output accumulators (SBUF, written by Pool)
            outacc = {
                qs: [_tt(pers, [HD, L], F32, f"out{qs}_{i}") for i in range(HPG)]
                for qs in (1, 2)
            }

            # ---- projection emitters ----
            def proj_k(ks, ots=(0, 1)):
                w = w_r[f"wk{ks}"]
                for ot in ots:
                    ps = _tt(stp, [128, L], F32, "st")
                    for dk in range(DT):
                        for nh in range(2):
                            nc.tensor.matmul(
                                ps[:, nh * 512 : (nh + 1) * 512],
                                w[dk][:, ot * 128 : (ot + 1) * 128],
                                x_r[ks][dk][:, nh * 512 : (nh + 1) * 512],
                                start=(dk == 0),
                                stop=(dk == DT - 1),
                            )
                    # scatter straight into kz: head 2*ot rows 0:64, head
                    # 2*ot+1 rows 64:128, each a contiguous [64, 1024] block.
                    for half in range(2):
                        hh = 2 * ot + half
                        po = half * 64
                        nc.vector.tensor_copy(
                            kz[ks][po : po + 64, hh * L : (hh + 1) * L],
                            ps[po : po + 64, :],
                        )

            def proj_q(qs, ots=(0, 1)):
                w = w_r[f"wq{qs}"]
                for ot in ots:
                    ps = _tt(stp, [128, L], F32, "st")
                    for dk in range(DT):
                        for nh in range(2):
                            nc.tensor.matmul(
                                ps[:, nh * 512 : (nh + 1) * 512],
                                w[dk][:, ot * 128 : (ot + 1) * 128],
                                x_r[qs][dk][:, nh * 512 : (nh + 1) * 512],
                                start=(dk == 0),
                                stop=(dk == DT - 1),
                            )
                    t = _tt(pers, [128, L], F16, f"q{qs}T_{ot}")
                    nc.vector.tensor_copy(t[:], ps[:])
                    qT[qs].append(t)

            def proj_v(side, lts):
                w = w_r[f"wv{side}"]
                for lt in lts:
                    ps = _tt(stp, [128, L], F32, "st")
                    for dk in range(DT):
                        nc.tensor.matmul(
                            ps[:, 0:OG],
                            x_r[side][dk][:, lt * 128 : (lt + 1) * 128],
                            w[dk][:],
                            start=(dk == 0),
                            stop=(dk == DT - 1),
                        )
                    t = _tt(pers, [128, HPG, HD + 1], BF16, f"v{side}_{lt}")
                    nc.vector.tensor_copy(
                        t[:, :, 0:HD], ps[:, 0:OG].rearrange("p (h d) -> p h d", h=HPG)
                    )
                    nc.vector.memset(t[:, :, HD : HD + 1], 1.0)
                    v_e[side].append(t)

            # ---- attention: one continuous software pipeline across all
            # (branch, kt) steps: QK(n) || exp(n-ish) || PV(n-1), so the PE
            # never sees a branch-boundary refill bubble. ----
            def norm_front(h, ks, acc):
                # free the acc banks quickly by copying [o; s] to SBUF (both
                # copies BEFORE the DMA-blocked reciprocal chain), then run
                # the reciprocal in a [128, 8] layout. fast=True (final
                # branch) replaces the two DMA-reshape round trips with a
                # direct [1, L] reciprocal_approx_fast (~18 correct bits) to
                # shorten the tail chain.
                oTs = {}
                srm = {}
                s128s = {}
                qlist = sorted(acc)
                for qs in qlist:
                    o = _tt(oTsp, [HD + 1, L], F32, "oTs")
                    nc.vector.tensor_copy(o[:], acc[qs][:])
                    oTs[qs] = o
                    s128 = _tt(smp, [128, KT], F32, "s128")
                    nc.sync.dma_start(s128[:], o[HD : HD + 1, :])
                    s128s[qs] = s128
                bcs = {}
                for qs in qlist:
                    r128 = _tt(smp, [128, KT], F32, "r128")
                    nc.vector.reciprocal(r128[:], s128s[qs][:])
                    rm128 = _tt(smp, [128, KT], F32, "rm128")
                    nc.vector.tensor_mul(rm128[:], r128[:], hm_sb[qs][:])
                    sr = _tt(smp, [1, L], F32, "srm")
                    nc.sync.dma_start(sr[:], rm128[:])
                    srm[qs] = sr
                    # broadcast 0.5*mask/s over 64 partitions on the idle
                    # Pool engine — keeps the broadcast off the PE and out
                    # of the PSUM st ring.
                    bc = _tt(smp, [HD, L], F32, "bc")
                    nc.gpsimd.partition_broadcast(bc[:], sr[:], channels=HD)
                    bcs[qs] = bc

                def finish():
                    # per-branch output mul/accumulate, qs=1 on DVE and
                    # qs=2 on Pool so the two chains run in parallel (all
                    # operands are SBUF now). Deferred to the next branch's
                    # kt5 so the s -> 1/s -> broadcast chain latency is
                    # hidden.
                    for qs in qlist:
                        oslice = outacc[qs][h][:]
                        if ks == 1:
                            nc.vector.tensor_mul(oslice, oTs[qs][0:HD, :], bcs[qs][:])
                        else:
                            t = _tt(tmpp, [HD, L], F32, "tmp")
                            nc.vector.tensor_mul(t[:], oTs[qs][0:HD, :], bcs[qs][:])
                            nc.vector.tensor_add(oslice, oslice, t[:])
                            nc.sync.dma_start(
                                {1: out1T, 2: out2T}[qs][h * HD : (h + 1) * HD, :],
                                oslice,
                            )

                return finish

            def attention(sched, fillers=()):
                # fillers: emission closures (deferred projection chunks)
                # injected at specific step indices into the pipeline's PE
                # slack. sched entries are (h, ks, qlist); the first real
                # branch is split into q1-only/q2-only passes so attention
                # starts right after the k1+q1 projections.
                accs = {}
                prev = None
                pending = None
                fill = dict(fillers)
                steps = [
                    (bi, h, ks, kt, qls)
                    for bi, (h, ks, qls) in enumerate(sched)
                    for kt in range(KT)
                ]
                for n, (bi, h, ks, kt, qls) in enumerate(steps):
                    if n in fill:
                        fill.pop(n)()
                    if kt == 0:
                        accs[bi] = {
                            qs: _tt(accp, [HD + 1, L], F32, "acc") for qs in qls
                        }
                    blk = h * KT + kt
                    lhsT = kz[ks][:, blk * 128 : (blk + 1) * 128]
                    sts = {}
                    for qs in qls:
                        st = _tt(stp, [128, L], F32, "st")
                        for nh in range(2):
                            nc.tensor.matmul(
                                st[:, nh * 512 : (nh + 1) * 512],
                                lhsT,
                                qT[qs][h // 2][:, nh * 512 : (nh + 1) * 512],
                                start=True,
                                stop=True,
                            )
                        sts[qs] = st
                    pts = {}
                    for qs in qls:
                        pt = _tt(ptp, [128, L], BF16, "pt")
                        nc.scalar.activation(
                            pt[:], sts[qs][:], EXP, bias=b_sb[ks][:, kt : kt + 1]
                        )
                        pts[qs] = pt
                    if prev is not None:
                        pending = _retire(prev, accs, pending)
                    prev = (bi, h, ks, kt, pts)
                pending = _retire(prev, accs, pending)
                if pending is not None:
                    pending()

            def _retire(prev, accs, pending):
                bi, h, ks, kt, pts = prev
                vt = v_e[ks][kt][:, h, :]
                for qs in pts:
                    for nh in range(2):
                        nc.tensor.matmul(
                            accs[bi][qs][:, nh * 512 : (nh + 1) * 512],
                            vt,
                            pts[qs][:, nh * 512 : (nh + 1) * 512],
                            start=(kt == 0),
                            stop=(kt == KT - 1),
                        )
                if kt == KT - 1:
                    return norm_front(h, ks, accs.pop(bi))
                if kt == 5 and pending is not None:
                    # mid-branch finish emission: the DVE queue has drained
                    # the previous norm chain by kt5, so the bc st-ring slots
                    # are consumed immediately, unlike at a branch boundary.
                    pending()
                    return None
                return pending

            # ---- emission schedule: ks=1 operands projected up front; the
            # ks=2 projections are injected into the attention pipeline's
            # slack at fixed step indices (all done well before step 32) ----
            proj_k(1)
            proj_q(1)
            proj_q(2)
            proj_v(1, range(KT))

            fillers = {2: lambda: proj_k(2, (0,)), 6: lambda: proj_k(2, (1,))}
            for j, lt in enumerate(range(KT)):
                fillers[10 + 2 * j] = lambda lt=lt: proj_v(2, (lt,))

            sched = [(h, ks, (1, 2)) for ks in (1, 2) for h in range(HPG)]
            attention(sched, fillers)

    nc.compile()
    return nc


def kernel(**inputs):
    global _NC
    if _NC is None:
        _NC = _build()

    input1 = np.asarray(inputs["input1"], dtype=np.float32)
    input2 = np.asarray(inputs["input2"], dtype=np.float32)
    mask1 = np.asarray(inputs["mask1"], dtype=np.float32)
    mask2 = np.asarray(inputs["mask2"], dtype=np.float32)
    W = {k: np.asarray(inputs[k], dtype=np.float32) for k in
         ("Wq1", "Wk1", "Wv1", "Wq2", "Wk2", "Wv2")}

    in_maps = []
    for core in range(8):
        b, hg = core // 2, core % 2
        og = slice(hg * OG, (hg + 1) * OG)
        m = {
            "x1T": np.ascontiguousarray(input1[b].T.astype(np.float16)),
            "x2T": np.ascontiguousarray(input2[b].T.astype(np.float16)),
            "bias1": np.ascontiguousarray(
                ((mask1[b] - 1.0) * INF).reshape(KT, 128).T
            ),
            "bias2": np.ascontiguousarray(
                ((mask2[b] - 1.0) * INF).reshape(KT, 128).T
            ),
            # [128, 8] layout matching the s-row DMA reshape (partition-major)
            "hm1": np.ascontiguousarray((0.5 * mask1[b]).reshape(128, KT)),
            "hm2": np.ascontiguousarray((0.5 * mask2[b]).reshape(128, KT)),
        }
        for wn in ("q1", "k1", "v1", "q2", "k2", "v2"):
            m["w" + wn] = np.ascontiguousarray(
                W["W" + wn[0] + wn[1]].T[:, og].astype(np.float16)
            )
        in_maps.append(m)

    global LAST_RESULT
    if TRACE:
        _install_ntff_hook()
    res = run_bass_kernel_spmd(_NC, in_maps, list(range(8)), trace=TRACE)
    LAST_RESULT = res

    output1 = np.empty((NB, L, D), dtype=np.float32)
    output2 = np.empty((NB, L, D), dtype=np.float32)
    for core in range(8):
        b, hg = core // 2, core % 2
        og = slice(hg * OG, (hg + 1) * OG)
        output1[b, :, og] = res.results[core]["out1T"].T
        output2[b, :, og] = res.results[core]["out2T"].T
    return (output1, output2)
